# revision 16
# baseline (speedup 1.0000x reference)
"""Causal self-attention (L=4096, D=1024, 16 heads) on 8 TRN2 NeuronCores.

Sharding: tensor-parallel over heads — each core owns 2 heads (128 head-dims).
Per core:
  QT/KT = W @ x.T (+bias)          [128, L]   (head-dims on partitions)
  VT    = Wv @ x.T (+bias)         [128, L]   then PE-transposed to V tiles
  S.T   = K @ Q.T  (per head)      [k, q] blocks, causal-skipped
  E     = exp(S.T/8) * mask        (no max-subtraction: |logits| < ~3.1)
  O.T   = [V|1].T @ E              -> unnormalized head outputs + col-sums
  O.T  /= sums  (PE broadcast + DVE reciprocal)
  partial = O @ Wo_slice.T         [L, D]
Host: out = sum_c(partial_c) + b_out.

All matmuls bf16 with fp32 PSUM accumulation.  Diagonal k-tiles narrow the
score matmul, exp, and mask to the not-fully-masked columns; fully-masked
columns live in dedicated always-zero e tiles.  Bulk PSUM->SBUF copies run
on the Pool engine to keep the DVE queue short.  Emission interleaves
projection work for chunk g+1 into the attention i-loop of chunk g so the
PE always has fill work while ACT (the exp bottleneck) drains, and the
normalize/out-projection epilogue of chunk g-1 is deferred into chunk g's
loop head.
"""

import numpy as np
import ml_dtypes

import concourse.bass as bass
import concourse.mybir as mybir
import concourse.tile as tile
from concourse import bacc
from concourse.bass import ts
from concourse.bass_utils import run_bass_kernel_spmd

L, D = 4096, 1024
P = 128
NCORES = 8
HDC = 128          # head-dims per core (2 heads x 64)
KO = D // P        # 8 contraction chunks of the model dim
NJ = L // 512      # 8 q-chunks of 512
NK = L // P        # 32 k-chunks of 128
BF16 = mybir.dt.bfloat16
F32 = mybir.dt.float32
EXP = mybir.ActivationFunctionType.Exp


def _build():
    nc = bacc.Bacc("TRN2", target_bir_lowering=False)

    xt_d = nc.dram_tensor("xt", [P, NJ, KO, 512], BF16, kind="ExternalInput")
    wq_d = nc.dram_tensor("wq", [P, KO, HDC], BF16, kind="ExternalInput")
    wk_d = nc.dram_tensor("wk", [P, KO, HDC], BF16, kind="ExternalInput")
    wv_d = nc.dram_tensor("wv", [P, KO, HDC], BF16, kind="ExternalInput")
    wo_d = nc.dram_tensor("wo", [HDC, D], BF16, kind="ExternalInput")
    bq_d = nc.dram_tensor("bq", [HDC, 1], F32, kind="ExternalInput")
    bk_d = nc.dram_tensor("bk", [HDC, 1], F32, kind="ExternalInput")
    bv_d = nc.dram_tensor("bv", [HDC, 1], F32, kind="ExternalInput")
    out_d = nc.dram_tensor("out", [L, D], BF16, kind="ExternalOutput")

    # [128,128] causal triangle for the diagonal 128-col sub-block:
    # mask[p, c] = 1 if c >= p  (k-position p may attend-from query c)
    qi = np.arange(P)
    mask_np = (qi[None, :] >= qi[:, None]).astype(ml_dtypes.bfloat16)
    mask_d = nc.inline_tensor(np.ascontiguousarray(mask_np), name="maskc")
    ident_np = np.eye(P, dtype=ml_dtypes.bfloat16)
    ident_d = nc.inline_tensor(np.ascontiguousarray(ident_np), name="identc")
    ones64_d = nc.inline_tensor(np.ones((1, 64), ml_dtypes.bfloat16), name="ones64c")

    with tile.TileContext(nc) as tc:
        with (
            tc.tile_pool(name="const", bufs=1) as cp,
            tc.tile_pool(name="work", bufs=4) as wp,
            tc.tile_pool(name="psum", bufs=1, space="PSUM") as pp,
        ):
            # ---- weights first, k-sliced, so the very first matmul can
            # start after ~64KB instead of ~1.6MB ----
            wq = cp.tile([P, KO, HDC], BF16, name="wq_s", tag="wq_s")
            wk = cp.tile([P, KO, HDC], BF16, name="wk_s", tag="wk_s")
            wv = cp.tile([P, KO, HDC], BF16, name="wv_s", tag="wv_s")
            xta = cp.tile([P, NJ, KO, 512], BF16, name="xta", tag="xta")
            maskt = cp.tile([P, P], BF16, name="mask_s", tag="mask_s")
            ident = cp.tile([P, P], BF16, name="ident_s", tag="ident_s")
            wo = cp.tile([P, D], BF16, name="wo_s", tag="wo_s")
            bq = cp.tile([P, 1], F32, name="bq_s", tag="bq_s")
            bk = cp.tile([P, 1], F32, name="bk_s", tag="bk_s")
            bv = cp.tile([P, 1], F32, name="bv_s", tag="bv_s")
            ones64 = cp.tile([1, 64], BF16, name="ones64_s", tag="ones64_s")

            # interleave wq halves with the matching x halves of token
            # group 0: the first Q-projection matmuls gate on ~0.8MB.
            # Tiny constants load after the first half — each dma_start
            # costs ~0.6us of issue time on the sync queue.
            nc.sync.dma_start(wq[:, 0:4], wq_d[:, 0:4])
            nc.sync.dma_start(xta[:, 0, 0:4], xt_d[:, 0, 0:4])
            nc.sync.dma_start(wq[:, 4:8], wq_d[:, 4:8])
            nc.sync.dma_start(xta[:, 0, 4:8], xt_d[:, 0, 4:8])
            nc.sync.dma_start(bq[:], bq_d[:])
            nc.sync.dma_start(wk[:], wk_d[:])
            nc.sync.dma_start(bk[:], bk_d[:])
            nc.sync.dma_start(xta[:, 1], xt_d[:, 1])
            nc.sync.dma_start(wv[:], wv_d[:])
            nc.sync.dma_start(bv[:], bv_d[:])
            nc.sync.dma_start(ident[:], ident_d[:])
            nc.sync.dma_start(maskt[:], mask_d[:])
            nc.sync.dma_start(ones64[:], ones64_d[:])
            nc.sync.dma_start(xta[:, 2], xt_d[:, 2])
            nc.sync.dma_start(wo[:], wo_d[:])
            for jcol in range(3, NJ):
                nc.sync.dma_start(xta[:, jcol], xt_d[:, jcol])

            qt = [cp.tile([P, 512], BF16, name=f"qt{j}", tag=f"qt{j}") for j in range(NJ)]
            kt = [cp.tile([P, 512], BF16, name=f"kt{j}", tag=f"kt{j}") for j in range(NJ)]
            ot = [cp.tile([P, 512], BF16, name=f"ot{j}", tag=f"ot{j}") for j in range(NJ)]
            # all V tiles: [tok-tile, head, head-dim (64) + ones col]
            v01 = cp.tile([P, NK, 2, 65], BF16, name="v01", tag="v01")
            nc.gpsimd.memset(v01[:, :, :, 64:65], 1.0)

            ppv = {}  # j -> (ppv0, ppv1) accumulation psums kept until epilogue

            def proj_qk(g, which):
                """QT or KT projection for token chunk g."""
                w, b, dst, nm = (wq, bq, qt, "q") if which == "q" else (wk, bk, kt, "k")
                ps = pp.tile([P, 512], F32, name=f"ps{nm}{g}", tag="mx", bufs=2)
                for k in range(KO):
                    nc.tensor.matmul(
                        ps[:], w[:, k, :], xta[:, g, k, :],
                        start=(k == 0), stop=(k == KO - 1),
                    )
                nc.vector.tensor_scalar_add(dst[g][:], ps[:], b[:])

            def proj_vt(g):
                """VT projection for token chunk g -> vt SBUF tile [hd, 512]."""
                psv = pp.tile([P, 512], F32, name=f"psv{g}", tag="mx", bufs=2)
                for k in range(KO):
                    nc.tensor.matmul(
                        psv[:], wv[:, k, :], xta[:, g, k, :],
                        start=(k == 0), stop=(k == KO - 1),
                    )
                vt = wp.tile([P, 512], BF16, name=f"vt{g}", tag="vt", bufs=2)
                nc.vector.tensor_scalar_add(vt[:], psv[:], bv[:])
                return vt

            def transp_v(g, vt):
                """Transpose chunk g's VT into v01 token tiles (PE + Pool)."""
                tr = pp.tile([P, 4, P], BF16, name=f"tr{g}", tag="mx", bufs=2)
                for t in range(4):
                    nc.tensor.transpose(tr[:, t, :], vt[:, ts(t, P)], ident[:])
                for h in range(2):
                    nc.vector.tensor_copy(
                        v01[:, 4 * g:4 * g + 4, h, 0:64], tr[:, :, ts(h, 64)]
                    )

            def normalize(j):
                """Normalize chunk j's head outputs into ot[j]."""
                ppv0, ppv1 = ppv.pop(j)
                s0 = wp.tile([1, 512], BF16, name=f"s0_{j}", tag="s0", bufs=4)
                s1 = wp.tile([1, 512], BF16, name=f"s1_{j}", tag="s1", bufs=4)
                nc.vector.tensor_copy(s0[:], ppv0[64:65, :])
                nc.vector.tensor_copy(s1[:], ppv1[64:65, :])
                pb = pp.tile([P, 512], F32, name=f"pb_{j}", tag="mx", bufs=2)
                nc.tensor.matmul(pb[0:64, :], ones64[:], s0[:], start=True, stop=True)
                nc.tensor.matmul(pb[64:128, :], ones64[:], s1[:], start=True, stop=True)
                rc = wp.tile([P, 512], F32, name=f"rc_{j}", tag="rc", bufs=4)
                nc.vector.reciprocal_approx_fast(rc[:], pb[:])
                nc.vector.tensor_mul(ot[j][0:64, :], ppv0[0:64, :], rc[0:64, :])
                nc.vector.tensor_mul(ot[j][64:128, :], ppv1[0:64, :], rc[64:128, :])

            def outproj(j, t, on_scalar=False):
                # alternate the two HWDGE rings (SP / Activation) so the
                # output drain runs on two DMA queues in parallel
                ob = wp.tile([P, D], BF16, name=f"ob_{t}", tag="ob", bufs=8)
                for n in range(2):
                    po = pp.tile([P, 512], F32, name=f"po_{t}_{n}", tag="mx", bufs=2)
                    nc.tensor.matmul(
                        po[:], ot[j][:, ts(t - 4 * j, P)], wo[:, ts(n, 512)],
                        start=True, stop=True,
                    )
                    nc.vector.tensor_copy(ob[:, ts(n, 512)], po[:])
                    if on_scalar:
                        # tail: issue each half as soon as it is copied, on
                        # its own ring (ACT stays free for the last exps)
                        ring = nc.sync if n == 0 else nc.scalar
                        ring.dma_start(out_d[ts(t, P), ts(n, 512)], ob[:, ts(n, 512)])
                if not on_scalar:
                    # early chunks: sync ring is still draining the x input,
                    # so route stores to the scalar ring; later alternate
                    ring = nc.scalar if (t < 16 or t % 2 == 1) else nc.sync
                    ring.dma_start(out_d[ts(t, P), :], ob[:])

            def final_slice(sl):
                """Normalize + out-project one 128-token slice of the last
                chunk, overlapped into its remaining diagonal i-iterations."""
                jf = NJ - 1
                fpv0, fpv1 = ppv[jf]
                c = ts(sl, P)
                s0 = wp.tile([1, P], BF16, name=f"s0f_{sl}", tag="s0f", bufs=4)
                s1 = wp.tile([1, P], BF16, name=f"s1f_{sl}", tag="s1f", bufs=4)
                nc.vector.tensor_copy(s0[:], fpv0[64:65, c])
                nc.vector.tensor_copy(s1[:], fpv1[64:65, c])
                pbf = pp.tile([P, P], F32, name=f"pbf_{sl}", tag="mx", bufs=2)
                nc.tensor.matmul(pbf[0:64, :], ones64[:], s0[:], start=True, stop=True)
                nc.tensor.matmul(pbf[64:128, :], ones64[:], s1[:], start=True, stop=True)
                rcf = wp.tile([P, P], F32, name=f"rcf_{sl}", tag="rcf", bufs=4)
                nc.vector.reciprocal_approx_fast(rcf[:], pbf[:])
                nc.vector.tensor_mul(ot[jf][0:64, c], fpv0[0:64, c], rcf[0:64, :])
                nc.vector.tensor_mul(ot[jf][64:128, c], fpv1[0:64, c], rcf[64:128, :])
                outproj(jf, 4 * jf + sl, on_scalar=True)

            # warm the PE p-state while the first DMAs land: ~10 dummy
            # matmuls on a memset tile keep the PE busy from t~0.3us so the
            # clock is ramped when real work arrives
            warm = cp.tile([P, 64], BF16, name="warm", tag="warm")
            nc.gpsimd.memset(warm[:], 0.0)
            for w in range(10):
                pw = pp.tile([P, 64], F32, name=f"pw{w}", tag="s", bufs=2)
                nc.tensor.matmul(pw[0:64, :], warm[0:64, :], warm[0:64, :],
                                 start=True, stop=True)

            # projections for chunk 0 up front
            proj_qk(0, "q")
            proj_qk(0, "k")
            vt0 = proj_vt(0)
            transp_v(0, vt0)

            for g in range(NJ):
                j = g
                nkj = 4 * (j + 1)
                ppv0 = pp.tile([65, 512], F32, name=f"ppv0_{j}", tag="ppv0", bufs=1)
                ppv1 = pp.tile([65, 512], F32, name=f"ppv1_{j}", tag="ppv1", bufs=1)
                ppv[j] = (ppv0, ppv1)

                # work units spread across this i-loop: projections for chunk
                # g+1, V tiles for chunk g+1, and the out-projection of the
                # already-normalized chunk g-2
                units = []
                if g + 1 < NJ:
                    units.append(lambda g=g: proj_qk(g + 1, "q"))
                    units.append(lambda g=g: proj_qk(g + 1, "k"))
                    vt_box = []
                    units.append(lambda g=g, b=vt_box: b.append(proj_vt(g + 1)))
                    units.append(lambda g=g, b=vt_box: transp_v(g + 1, b.pop()))
                if g >= 2:
                    for t in range(4 * (g - 2), 4 * (g - 2) + 4):
                        units.append(lambda t=t, g=g: outproj(g - 2, t))
                if g == NJ - 1:
                    for t in range(4 * (g - 1), 4 * (g - 1) + 4):
                        units.append(lambda t=t, g=g: outproj(g - 1, t))
                nu = len(units)
                slots = {}
                for u in range(nu):
                    slots.setdefault(min(nkj - 1, 1 + (u * nkj) // (nu + 1)), []).append(units[u])

                for i in range(nkj):
                    m = i - 4 * j
                    ps = pp.tile([P, 2, 512], F32, name=f"ps_{j}_{i}", tag="s", bufs=2)
                    c0 = 128 * m if m > 0 else 0
                    for h in range(2):
                        nc.tensor.matmul(
                            ps[:, h, c0:512],
                            kt[i // 4][ts(h, 64), ts(i % 4, P)],
                            qt[j][ts(h, 64), c0:512],
                            start=True, stop=True,
                        )
                    e = wp.tile([P, 2, 512], BF16, name=f"e_{j}_{i}", tag="e", bufs=8)
                    nc.scalar.activation(e[:, :, c0:512], ps[:, :, c0:512], EXP, scale=0.125)
                    if m >= 0:
                        for h in range(2):
                            nc.vector.tensor_mul(
                                e[:, h, c0:c0 + 128], e[:, h, c0:c0 + 128], maskt[:]
                            )
                    if i == 0 and j > 0:
                        normalize(j - 1)
                    # diagonal tiles only contribute to query columns >= c0;
                    # the rest of the accumulator is left untouched
                    nc.tensor.matmul(
                        ppv0[:, c0:512], v01[:, i, 0, :], e[:, 0, c0:512],
                        start=(i == 0), stop=(i == nkj - 1),
                        skip_group_check=True,
                    )
                    nc.tensor.matmul(
                        ppv1[:, c0:512], v01[:, i, 1, :], e[:, 1, c0:512],
                        start=(i == 0), stop=(i == nkj - 1),
                        skip_group_check=True,
                    )
                    # last chunk: each diagonal step completes one 128-token
                    # slice; normalize + out-project it while later diagonal
                    # steps still run
                    if j == NJ - 1 and m >= 1:
                        final_slice(m - 1)
                    for fn in slots.get(i, []):
                        fn()

            final_slice(3)
            ppv.pop(NJ - 1)

    nc.compile()
    return nc


def _pack_w(w_slice):
    """[HDC, D] weight slice -> transposed, chunked [P, KO, HDC] bf16."""
    wt = np.ascontiguousarray(w_slice.T)          # [D, HDC]
    return np.ascontiguousarray(
        wt.reshape(KO, P, HDC).transpose(1, 0, 2)
    ).astype(ml_dtypes.bfloat16)


def _make_in_maps(x, W_qkv, b_qkv, W_out, b_out):
    bf = ml_dtypes.bfloat16
    x = np.asarray(x, np.float32)
    W_qkv = np.asarray(W_qkv, np.float32)
    b_qkv = np.asarray(b_qkv, np.float32)
    W_out = np.asarray(W_out, np.float32)
    xt = np.ascontiguousarray(
        x.T.reshape(KO, P, NJ, 512).transpose(1, 2, 0, 3)
    ).astype(bf)
    in_maps = []
    for c in range(NCORES):
        r = slice(HDC * c, HDC * (c + 1))
        in_maps.append({
            "xt": xt,
            "wq": _pack_w(W_qkv[0 * D:1 * D][r]),
            "wk": _pack_w(W_qkv[1 * D:2 * D][r]),
            "wv": _pack_w(W_qkv[2 * D:3 * D][r]),
            "wo": np.ascontiguousarray(W_out[:, r].T).astype(bf),
            "bq": np.ascontiguousarray(b_qkv[0 * D:1 * D][r][:, None]).astype(np.float32),
            "bk": np.ascontiguousarray(b_qkv[1 * D:2 * D][r][:, None]).astype(np.float32),
            "bv": np.ascontiguousarray(b_qkv[2 * D:3 * D][r][:, None]).astype(np.float32),
        })
    return in_maps


_NC_CACHE = {}


def kernel(x, W_qkv, b_qkv, W_out, b_out):
    if "nc" not in _NC_CACHE:
        _NC_CACHE["nc"] = _build()
    nc = _NC_CACHE["nc"]
    in_maps = _make_in_maps(x, W_qkv, b_qkv, W_out, b_out)
    res = run_bass_kernel_spmd(nc, in_maps, core_ids=list(range(NCORES)))
    out = np.zeros((L, D), np.float32)
    for c in range(NCORES):
        out += res.results[c]["out"].astype(np.float32)
    out += np.asarray(b_out, np.float32)[None, :]
    return out


# revision 18
# speedup vs baseline: 1.0104x; 1.0104x over previous
"""Causal self-attention (L=4096, D=1024, 16 heads) on 8 TRN2 NeuronCores.

Sharding: tensor-parallel over heads — each core owns 2 heads (128 head-dims).
Per core:
  QT/KT = W @ x.T (+bias)          [128, L]   (head-dims on partitions)
  VT    = Wv @ x.T (+bias)         [128, L]   then PE-transposed to V tiles
  S.T   = K @ Q.T  (per head)      [k, q] blocks, causal-skipped
  E     = exp(S.T/8) * mask        (no max-subtraction: |logits| < ~3.1)
  O.T   = [V|1].T @ E              -> unnormalized head outputs + col-sums
  O.T  /= sums  (PE broadcast + DVE reciprocal)
  partial = O @ Wo_slice.T         [L, D]
Host: out = sum_c(partial_c) + b_out.

All matmuls bf16 with fp32 PSUM accumulation.  Diagonal k-tiles narrow the
score matmul, exp, and mask to the not-fully-masked columns; fully-masked
columns live in dedicated always-zero e tiles.  Bulk PSUM->SBUF copies run
on the Pool engine to keep the DVE queue short.  Emission interleaves
projection work for chunk g+1 into the attention i-loop of chunk g so the
PE always has fill work while ACT (the exp bottleneck) drains, and the
normalize/out-projection epilogue of chunk g-1 is deferred into chunk g's
loop head.
"""

import numpy as np
import ml_dtypes

import concourse.bass as bass
import concourse.mybir as mybir
import concourse.tile as tile
from concourse import bacc
from concourse.bass import ts
from concourse.bass_utils import run_bass_kernel_spmd

L, D = 4096, 1024
P = 128
NCORES = 8
HDC = 128          # head-dims per core (2 heads x 64)
KO = D // P        # 8 contraction chunks of the model dim
NJ = L // 512      # 8 q-chunks of 512
NK = L // P        # 32 k-chunks of 128
BF16 = mybir.dt.bfloat16
F32 = mybir.dt.float32
EXP = mybir.ActivationFunctionType.Exp


def _build():
    nc = bacc.Bacc("TRN2", target_bir_lowering=False)

    xt_d = nc.dram_tensor("xt", [P, NJ, KO, 512], BF16, kind="ExternalInput")
    wq_d = nc.dram_tensor("wq", [P, KO, HDC], BF16, kind="ExternalInput")
    wk_d = nc.dram_tensor("wk", [P, KO, HDC], BF16, kind="ExternalInput")
    wv_d = nc.dram_tensor("wv", [P, KO, HDC], BF16, kind="ExternalInput")
    wo_d = nc.dram_tensor("wo", [HDC, D], BF16, kind="ExternalInput")
    bq_d = nc.dram_tensor("bq", [HDC, 1], F32, kind="ExternalInput")
    bk_d = nc.dram_tensor("bk", [HDC, 1], F32, kind="ExternalInput")
    bv_d = nc.dram_tensor("bv", [HDC, 1], F32, kind="ExternalInput")
    out_d = nc.dram_tensor("out", [L, D], BF16, kind="ExternalOutput")

    # [128,128] causal triangle for the diagonal 128-col sub-block:
    # mask[p, c] = 1 if c >= p  (k-position p may attend-from query c)
    qi = np.arange(P)
    mask_np = (qi[None, :] >= qi[:, None]).astype(ml_dtypes.bfloat16)
    mask_d = nc.inline_tensor(np.ascontiguousarray(mask_np), name="maskc")
    ident_np = np.eye(P, dtype=ml_dtypes.bfloat16)
    ident_d = nc.inline_tensor(np.ascontiguousarray(ident_np), name="identc")
    ones64_d = nc.inline_tensor(np.ones((1, 64), ml_dtypes.bfloat16), name="ones64c")

    with tile.TileContext(nc) as tc:
        with (
            tc.tile_pool(name="const", bufs=1) as cp,
            tc.tile_pool(name="work", bufs=4) as wp,
            tc.tile_pool(name="psum", bufs=1, space="PSUM") as pp,
        ):
            # ---- weights first, k-sliced, so the very first matmul can
            # start after ~64KB instead of ~1.6MB ----
            wq = cp.tile([P, KO, HDC], BF16, name="wq_s", tag="wq_s")
            wk = cp.tile([P, KO, HDC], BF16, name="wk_s", tag="wk_s")
            wv = cp.tile([P, KO, HDC], BF16, name="wv_s", tag="wv_s")
            xta = cp.tile([P, NJ, KO, 512], BF16, name="xta", tag="xta")
            maskt = cp.tile([P, P], BF16, name="mask_s", tag="mask_s")
            ident = cp.tile([P, P], BF16, name="ident_s", tag="ident_s")
            wo = cp.tile([P, D], BF16, name="wo_s", tag="wo_s")
            bq = cp.tile([P, 1], F32, name="bq_s", tag="bq_s")
            bk = cp.tile([P, 1], F32, name="bk_s", tag="bk_s")
            bv = cp.tile([P, 1], F32, name="bv_s", tag="bv_s")
            ones64 = cp.tile([1, 64], BF16, name="ones64_s", tag="ones64_s")

            # interleave wq halves with the matching x halves of token
            # group 0: the first Q-projection matmuls gate on ~0.8MB.
            # Tiny constants load after the first half — each dma_start
            # costs ~0.6us of issue time on the sync queue.
            nc.sync.dma_start(wq[:, 0:4], wq_d[:, 0:4])
            nc.sync.dma_start(xta[:, 0, 0:4], xt_d[:, 0, 0:4])
            nc.sync.dma_start(wq[:, 4:8], wq_d[:, 4:8])
            nc.sync.dma_start(xta[:, 0, 4:8], xt_d[:, 0, 4:8])
            nc.sync.dma_start(bq[:], bq_d[:])
            nc.sync.dma_start(wk[:], wk_d[:])
            nc.sync.dma_start(bk[:], bk_d[:])
            nc.sync.dma_start(xta[:, 1], xt_d[:, 1])
            nc.sync.dma_start(wv[:], wv_d[:])
            nc.sync.dma_start(bv[:], bv_d[:])
            nc.sync.dma_start(ident[:], ident_d[:])
            nc.sync.dma_start(maskt[:], mask_d[:])
            nc.sync.dma_start(ones64[:], ones64_d[:])
            nc.sync.dma_start(xta[:, 2], xt_d[:, 2])
            nc.sync.dma_start(wo[:], wo_d[:])
            for jcol in range(3, NJ):
                nc.sync.dma_start(xta[:, jcol], xt_d[:, jcol])

            qt = [cp.tile([P, 512], BF16, name=f"qt{j}", tag=f"qt{j}") for j in range(NJ)]
            kt = [cp.tile([P, 512], BF16, name=f"kt{j}", tag=f"kt{j}") for j in range(NJ)]
            ot = [cp.tile([P, 512], BF16, name=f"ot{j}", tag=f"ot{j}") for j in range(NJ)]
            # all V tiles: [tok-tile, head, head-dim (64) + ones col]
            v01 = cp.tile([P, NK, 2, 65], BF16, name="v01", tag="v01")
            nc.gpsimd.memset(v01[:, :, :, 64:65], 1.0)

            ppv = {}  # j -> (ppv0, ppv1) accumulation psums kept until epilogue

            def proj_qk(g, which):
                """QT or KT projection for token chunk g."""
                w, b, dst, nm = (wq, bq, qt, "q") if which == "q" else (wk, bk, kt, "k")
                ps = pp.tile([P, 512], F32, name=f"ps{nm}{g}", tag="mx", bufs=2)
                for k in range(KO):
                    nc.tensor.matmul(
                        ps[:], w[:, k, :], xta[:, g, k, :],
                        start=(k == 0), stop=(k == KO - 1),
                    )
                nc.vector.tensor_scalar_add(dst[g][:], ps[:], b[:])

            def proj_vt(g):
                """VT projection for token chunk g -> vt SBUF tile [hd, 512]."""
                psv = pp.tile([P, 512], F32, name=f"psv{g}", tag="mx", bufs=2)
                for k in range(KO):
                    nc.tensor.matmul(
                        psv[:], wv[:, k, :], xta[:, g, k, :],
                        start=(k == 0), stop=(k == KO - 1),
                    )
                vt = wp.tile([P, 512], BF16, name=f"vt{g}", tag="vt", bufs=2)
                nc.vector.tensor_scalar_add(vt[:], psv[:], bv[:])
                return vt

            def transp_v(g, vt):
                """Transpose chunk g's VT into v01 token tiles (PE + Pool)."""
                tr = pp.tile([P, 4, P], BF16, name=f"tr{g}", tag="mx", bufs=2)
                for t in range(4):
                    nc.tensor.transpose(tr[:, t, :], vt[:, ts(t, P)], ident[:])
                for h in range(2):
                    nc.vector.tensor_copy(
                        v01[:, 4 * g:4 * g + 4, h, 0:64], tr[:, :, ts(h, 64)]
                    )

            def normalize(j):
                """Normalize chunk j's head outputs into ot[j]."""
                ppv0, ppv1 = ppv.pop(j)
                s0 = wp.tile([1, 512], BF16, name=f"s0_{j}", tag="s0", bufs=4)
                s1 = wp.tile([1, 512], BF16, name=f"s1_{j}", tag="s1", bufs=4)
                nc.vector.tensor_copy(s0[:], ppv0[64:65, :])
                nc.vector.tensor_copy(s1[:], ppv1[64:65, :])
                pb = pp.tile([P, 512], F32, name=f"pb_{j}", tag="mx", bufs=2)
                nc.tensor.matmul(pb[0:64, :], ones64[:], s0[:], start=True, stop=True)
                nc.tensor.matmul(pb[64:128, :], ones64[:], s1[:], start=True, stop=True)
                rc = wp.tile([P, 512], F32, name=f"rc_{j}", tag="rc", bufs=4)
                nc.vector.reciprocal_approx_fast(rc[:], pb[:])
                nc.vector.tensor_mul(ot[j][0:64, :], ppv0[0:64, :], rc[0:64, :])
                nc.vector.tensor_mul(ot[j][64:128, :], ppv1[0:64, :], rc[64:128, :])

            def outproj(j, t, on_scalar=False):
                # alternate the two HWDGE rings (SP / Activation) so the
                # output drain runs on two DMA queues in parallel
                ob = wp.tile([P, D], BF16, name=f"ob_{t}", tag="ob", bufs=8)
                for n in range(2):
                    po = pp.tile([P, 512], F32, name=f"po_{t}_{n}", tag="mx", bufs=2)
                    nc.tensor.matmul(
                        po[:], ot[j][:, ts(t - 4 * j, P)], wo[:, ts(n, 512)],
                        start=True, stop=True,
                    )
                    if on_scalar:
                        nc.scalar.copy(ob[:, ts(n, 512)], po[:])
                    else:
                        nc.vector.tensor_copy(ob[:, ts(n, 512)], po[:])
                nc.sync.dma_start(out_d[ts(t, P), :], ob[:])

            def final_slice(sl):
                """Normalize + out-project one 128-token slice of the last
                chunk, overlapped into its remaining diagonal i-iterations."""
                jf = NJ - 1
                fpv0, fpv1 = ppv[jf]
                c = ts(sl, P)
                s0 = wp.tile([1, P], BF16, name=f"s0f_{sl}", tag="s0f", bufs=4)
                s1 = wp.tile([1, P], BF16, name=f"s1f_{sl}", tag="s1f", bufs=4)
                nc.vector.tensor_copy(s0[:], fpv0[64:65, c])
                nc.vector.tensor_copy(s1[:], fpv1[64:65, c])
                pbf = pp.tile([P, P], F32, name=f"pbf_{sl}", tag="mx", bufs=2)
                nc.tensor.matmul(pbf[0:64, :], ones64[:], s0[:], start=True, stop=True)
                nc.tensor.matmul(pbf[64:128, :], ones64[:], s1[:], start=True, stop=True)
                rcf = wp.tile([P, P], F32, name=f"rcf_{sl}", tag="rcf", bufs=4)
                nc.vector.reciprocal_approx_fast(rcf[:], pbf[:])
                nc.vector.tensor_mul(ot[jf][0:64, c], fpv0[0:64, c], rcf[0:64, :])
                nc.vector.tensor_mul(ot[jf][64:128, c], fpv1[0:64, c], rcf[64:128, :])
                outproj(jf, 4 * jf + sl, on_scalar=True)

            # warm the PE p-state while the first DMAs land: ~10 dummy
            # matmuls on a memset tile keep the PE busy from t~0.3us so the
            # clock is ramped when real work arrives
            warm = cp.tile([P, 64], BF16, name="warm", tag="warm")
            nc.gpsimd.memset(warm[:], 0.0)
            for w in range(10):
                pw = pp.tile([P, 64], F32, name=f"pw{w}", tag="s", bufs=2)
                nc.tensor.matmul(pw[0:64, :], warm[0:64, :], warm[0:64, :],
                                 start=True, stop=True)

            def warmups(n0, n):
                # dependency-free matmuls the PE bypass queue can run while
                # the projection matmuls wait on their DMAs, keeping the
                # p-state ramped through the startup stalls
                for w in range(n0, n0 + n):
                    pw = pp.tile([P, 64], F32, name=f"pw{w}", tag="s", bufs=2)
                    nc.tensor.matmul(pw[0:64, :], warm[0:64, :], warm[0:64, :],
                                     start=True, stop=True)

            # projections for chunk 0 up front
            proj_qk(0, "q")
            warmups(10, 6)
            proj_qk(0, "k")
            warmups(16, 6)
            vt0 = proj_vt(0)
            transp_v(0, vt0)

            for g in range(NJ):
                j = g
                nkj = 4 * (j + 1)
                ppv0 = pp.tile([65, 512], F32, name=f"ppv0_{j}", tag="ppv0", bufs=1)
                ppv1 = pp.tile([65, 512], F32, name=f"ppv1_{j}", tag="ppv1", bufs=1)
                ppv[j] = (ppv0, ppv1)

                # work units spread across this i-loop: projections for chunk
                # g+1, V tiles for chunk g+1, and the out-projection of the
                # already-normalized chunk g-2
                units = []
                if g + 1 < NJ:
                    units.append(lambda g=g: proj_qk(g + 1, "q"))
                    units.append(lambda g=g: proj_qk(g + 1, "k"))
                    vt_box = []
                    units.append(lambda g=g, b=vt_box: b.append(proj_vt(g + 1)))
                    units.append(lambda g=g, b=vt_box: transp_v(g + 1, b.pop()))
                if g >= 2:
                    for t in range(4 * (g - 2), 4 * (g - 2) + 4):
                        units.append(lambda t=t, g=g: outproj(g - 2, t))
                if g == NJ - 1:
                    for t in range(4 * (g - 1), 4 * (g - 1) + 4):
                        units.append(lambda t=t, g=g: outproj(g - 1, t))
                nu = len(units)
                slots = {}
                for u in range(nu):
                    slots.setdefault(min(nkj - 1, 1 + (u * nkj) // (nu + 1)), []).append(units[u])

                for i in range(nkj):
                    m = i - 4 * j
                    ps = pp.tile([P, 2, 512], F32, name=f"ps_{j}_{i}", tag="s", bufs=2)
                    c0 = 128 * m if m > 0 else 0
                    for h in range(2):
                        nc.tensor.matmul(
                            ps[:, h, c0:512],
                            kt[i // 4][ts(h, 64), ts(i % 4, P)],
                            qt[j][ts(h, 64), c0:512],
                            start=True, stop=True,
                        )
                    e = wp.tile([P, 2, 512], BF16, name=f"e_{j}_{i}", tag="e", bufs=8)
                    nc.scalar.activation(e[:, :, c0:512], ps[:, :, c0:512], EXP, scale=0.125)
                    if m >= 0:
                        for h in range(2):
                            nc.vector.tensor_mul(
                                e[:, h, c0:c0 + 128], e[:, h, c0:c0 + 128], maskt[:]
                            )
                    if i == 0 and j > 0:
                        normalize(j - 1)
                    # diagonal tiles only contribute to query columns >= c0;
                    # the rest of the accumulator is left untouched
                    nc.tensor.matmul(
                        ppv0[:, c0:512], v01[:, i, 0, :], e[:, 0, c0:512],
                        start=(i == 0), stop=(i == nkj - 1),
                        skip_group_check=True,
                    )
                    nc.tensor.matmul(
                        ppv1[:, c0:512], v01[:, i, 1, :], e[:, 1, c0:512],
                        start=(i == 0), stop=(i == nkj - 1),
                        skip_group_check=True,
                    )
                    # last chunk: each diagonal step completes one 128-token
                    # slice; normalize + out-project it while later diagonal
                    # steps still run
                    if j == NJ - 1 and m >= 1:
                        final_slice(m - 1)
                    for fn in slots.get(i, []):
                        fn()

            final_slice(3)
            ppv.pop(NJ - 1)

    nc.compile()
    return nc


def _pack_w(w_slice):
    """[HDC, D] weight slice -> transposed, chunked [P, KO, HDC] bf16."""
    wt = np.ascontiguousarray(w_slice.T)          # [D, HDC]
    return np.ascontiguousarray(
        wt.reshape(KO, P, HDC).transpose(1, 0, 2)
    ).astype(ml_dtypes.bfloat16)


def _make_in_maps(x, W_qkv, b_qkv, W_out, b_out):
    bf = ml_dtypes.bfloat16
    x = np.asarray(x, np.float32)
    W_qkv = np.asarray(W_qkv, np.float32)
    b_qkv = np.asarray(b_qkv, np.float32)
    W_out = np.asarray(W_out, np.float32)
    xt = np.ascontiguousarray(
        x.T.reshape(KO, P, NJ, 512).transpose(1, 2, 0, 3)
    ).astype(bf)
    in_maps = []
    for c in range(NCORES):
        r = slice(HDC * c, HDC * (c + 1))
        in_maps.append({
            "xt": xt,
            "wq": _pack_w(W_qkv[0 * D:1 * D][r]),
            "wk": _pack_w(W_qkv[1 * D:2 * D][r]),
            "wv": _pack_w(W_qkv[2 * D:3 * D][r]),
            "wo": np.ascontiguousarray(W_out[:, r].T).astype(bf),
            "bq": np.ascontiguousarray(b_qkv[0 * D:1 * D][r][:, None]).astype(np.float32),
            "bk": np.ascontiguousarray(b_qkv[1 * D:2 * D][r][:, None]).astype(np.float32),
            "bv": np.ascontiguousarray(b_qkv[2 * D:3 * D][r][:, None]).astype(np.float32),
        })
    return in_maps


_NC_CACHE = {}


def kernel(x, W_qkv, b_qkv, W_out, b_out):
    if "nc" not in _NC_CACHE:
        _NC_CACHE["nc"] = _build()
    nc = _NC_CACHE["nc"]
    in_maps = _make_in_maps(x, W_qkv, b_qkv, W_out, b_out)
    res = run_bass_kernel_spmd(nc, in_maps, core_ids=list(range(NCORES)))
    out = np.zeros((L, D), np.float32)
    for c in range(NCORES):
        out += res.results[c]["out"].astype(np.float32)
    out += np.asarray(b_out, np.float32)[None, :]
    return out


# revision 19
# speedup vs baseline: 1.0158x; 1.0053x over previous
"""Causal self-attention (L=4096, D=1024, 16 heads) on 8 TRN2 NeuronCores.

Sharding: tensor-parallel over heads — each core owns 2 heads (128 head-dims).
Per core:
  QT/KT = W @ x.T (+bias)          [128, L]   (head-dims on partitions)
  VT    = Wv @ x.T (+bias)         [128, L]   then PE-transposed to V tiles
  S.T   = K @ Q.T  (per head)      [k, q] blocks, causal-skipped
  E     = exp(S.T/8) * mask        (no max-subtraction: |logits| < ~3.1)
  O.T   = [V|1].T @ E              -> unnormalized head outputs + col-sums
  O.T  /= sums  (PE broadcast + DVE reciprocal)
  partial = O @ Wo_slice.T         [L, D]
Host: out = sum_c(partial_c) + b_out.

All matmuls bf16 with fp32 PSUM accumulation.  Diagonal k-tiles narrow the
score matmul, exp, and mask to the not-fully-masked columns; fully-masked
columns live in dedicated always-zero e tiles.  Bulk PSUM->SBUF copies run
on the Pool engine to keep the DVE queue short.  Emission interleaves
projection work for chunk g+1 into the attention i-loop of chunk g so the
PE always has fill work while ACT (the exp bottleneck) drains, and the
normalize/out-projection epilogue of chunk g-1 is deferred into chunk g's
loop head.
"""

import numpy as np
import ml_dtypes

import concourse.bass as bass
import concourse.mybir as mybir
import concourse.tile as tile
from concourse import bacc
from concourse.bass import ts
from concourse.bass_utils import run_bass_kernel_spmd

L, D = 4096, 1024
P = 128
NCORES = 8
HDC = 128          # head-dims per core (2 heads x 64)
KO = D // P        # 8 contraction chunks of the model dim
NJ = L // 512      # 8 q-chunks of 512
NK = L // P        # 32 k-chunks of 128
BF16 = mybir.dt.bfloat16
F32 = mybir.dt.float32
EXP = mybir.ActivationFunctionType.Exp


def _build():
    nc = bacc.Bacc("TRN2", target_bir_lowering=False)

    xt_d = nc.dram_tensor("xt", [P, NJ, KO, 512], BF16, kind="ExternalInput")
    wq_d = nc.dram_tensor("wq", [P, KO, HDC], BF16, kind="ExternalInput")
    wk_d = nc.dram_tensor("wk", [P, KO, HDC], BF16, kind="ExternalInput")
    wv_d = nc.dram_tensor("wv", [P, KO, HDC], BF16, kind="ExternalInput")
    wo_d = nc.dram_tensor("wo", [HDC, D], BF16, kind="ExternalInput")
    bq_d = nc.dram_tensor("bq", [HDC, 1], F32, kind="ExternalInput")
    bk_d = nc.dram_tensor("bk", [HDC, 1], F32, kind="ExternalInput")
    bv_d = nc.dram_tensor("bv", [HDC, 1], F32, kind="ExternalInput")
    out_d = nc.dram_tensor("out", [L, D], BF16, kind="ExternalOutput")

    # [128,128] causal triangle for the diagonal 128-col sub-block:
    # mask[p, c] = 1 if c >= p  (k-position p may attend-from query c)
    qi = np.arange(P)
    mask_np = (qi[None, :] >= qi[:, None]).astype(ml_dtypes.bfloat16)
    mask_d = nc.inline_tensor(np.ascontiguousarray(mask_np), name="maskc")
    ident_np = np.eye(P, dtype=ml_dtypes.bfloat16)
    ident_d = nc.inline_tensor(np.ascontiguousarray(ident_np), name="identc")
    ones64_d = nc.inline_tensor(np.ones((1, 64), ml_dtypes.bfloat16), name="ones64c")

    with tile.TileContext(nc) as tc:
        with (
            tc.tile_pool(name="const", bufs=1) as cp,
            tc.tile_pool(name="work", bufs=4) as wp,
            tc.tile_pool(name="psum", bufs=1, space="PSUM") as pp,
        ):
            # ---- weights first, k-sliced, so the very first matmul can
            # start after ~64KB instead of ~1.6MB ----
            wq = cp.tile([P, KO, HDC], BF16, name="wq_s", tag="wq_s")
            wk = cp.tile([P, KO, HDC], BF16, name="wk_s", tag="wk_s")
            wv = cp.tile([P, KO, HDC], BF16, name="wv_s", tag="wv_s")
            xta = cp.tile([P, NJ, KO, 512], BF16, name="xta", tag="xta")
            maskt = cp.tile([P, P], BF16, name="mask_s", tag="mask_s")
            ident = cp.tile([P, P], BF16, name="ident_s", tag="ident_s")
            wo = cp.tile([P, D], BF16, name="wo_s", tag="wo_s")
            bq = cp.tile([P, 1], F32, name="bq_s", tag="bq_s")
            bk = cp.tile([P, 1], F32, name="bk_s", tag="bk_s")
            bv = cp.tile([P, 1], F32, name="bv_s", tag="bv_s")
            ones64 = cp.tile([1, 64], BF16, name="ones64_s", tag="ones64_s")

            # interleave wq halves with the matching x halves of token
            # group 0: the first Q-projection matmuls gate on ~0.8MB.
            # Tiny constants load after the first half — each dma_start
            # costs ~0.6us of issue time on the sync queue.
            nc.sync.dma_start(wq[:, 0:4], wq_d[:, 0:4])
            nc.sync.dma_start(xta[:, 0, 0:4], xt_d[:, 0, 0:4])
            nc.sync.dma_start(wq[:, 4:8], wq_d[:, 4:8])
            nc.sync.dma_start(xta[:, 0, 4:8], xt_d[:, 0, 4:8])
            nc.sync.dma_start(bq[:], bq_d[:])
            nc.sync.dma_start(wk[:], wk_d[:])
            nc.sync.dma_start(bk[:], bk_d[:])
            nc.sync.dma_start(xta[:, 1], xt_d[:, 1])
            nc.sync.dma_start(wv[:], wv_d[:])
            nc.sync.dma_start(bv[:], bv_d[:])
            nc.sync.dma_start(ident[:], ident_d[:])
            nc.sync.dma_start(maskt[:], mask_d[:])
            nc.sync.dma_start(ones64[:], ones64_d[:])
            nc.sync.dma_start(xta[:, 2], xt_d[:, 2])
            nc.sync.dma_start(wo[:], wo_d[:])
            for jcol in range(3, NJ):
                nc.sync.dma_start(xta[:, jcol], xt_d[:, jcol])

            qt = [cp.tile([P, 512], BF16, name=f"qt{j}", tag=f"qt{j}") for j in range(NJ)]
            kt = [cp.tile([P, 512], BF16, name=f"kt{j}", tag=f"kt{j}") for j in range(NJ)]
            ot = [cp.tile([P, 512], BF16, name=f"ot{j}", tag=f"ot{j}") for j in range(NJ)]
            # all V tiles: [tok-tile, head, head-dim (64) + ones col]
            v01 = cp.tile([P, NK, 2, 65], BF16, name="v01", tag="v01")
            nc.gpsimd.memset(v01[:, :, :, 64:65], 1.0)

            ppv = {}  # j -> (ppv0, ppv1) accumulation psums kept until epilogue

            def proj_qk(g, which):
                """QT or KT projection for token chunk g."""
                w, b, dst, nm = (wq, bq, qt, "q") if which == "q" else (wk, bk, kt, "k")
                ps = pp.tile([P, 512], F32, name=f"ps{nm}{g}", tag="mx", bufs=2)
                for k in range(KO):
                    nc.tensor.matmul(
                        ps[:], w[:, k, :], xta[:, g, k, :],
                        start=(k == 0), stop=(k == KO - 1),
                    )
                nc.vector.tensor_scalar_add(dst[g][:], ps[:], b[:])

            def proj_vt(g):
                """VT projection for token chunk g -> vt SBUF tile [hd, 512]."""
                psv = pp.tile([P, 512], F32, name=f"psv{g}", tag="mx", bufs=2)
                for k in range(KO):
                    nc.tensor.matmul(
                        psv[:], wv[:, k, :], xta[:, g, k, :],
                        start=(k == 0), stop=(k == KO - 1),
                    )
                vt = wp.tile([P, 512], BF16, name=f"vt{g}", tag="vt", bufs=2)
                nc.vector.tensor_scalar_add(vt[:], psv[:], bv[:])
                return vt

            def transp_v(g, vt):
                """Transpose chunk g's VT into v01 token tiles (PE + Pool)."""
                tr = pp.tile([P, 4, P], BF16, name=f"tr{g}", tag="mx", bufs=2)
                for t in range(4):
                    nc.tensor.transpose(tr[:, t, :], vt[:, ts(t, P)], ident[:])
                for h in range(2):
                    nc.vector.tensor_copy(
                        v01[:, 4 * g:4 * g + 4, h, 0:64], tr[:, :, ts(h, 64)]
                    )

            def normalize(j):
                """Normalize chunk j's head outputs into ot[j]."""
                ppv0, ppv1 = ppv.pop(j)
                s0 = wp.tile([1, 512], BF16, name=f"s0_{j}", tag="s0", bufs=4)
                s1 = wp.tile([1, 512], BF16, name=f"s1_{j}", tag="s1", bufs=4)
                nc.vector.tensor_copy(s0[:], ppv0[64:65, :])
                nc.vector.tensor_copy(s1[:], ppv1[64:65, :])
                pb = pp.tile([P, 512], F32, name=f"pb_{j}", tag="mx", bufs=2)
                nc.tensor.matmul(pb[0:64, :], ones64[:], s0[:], start=True, stop=True)
                nc.tensor.matmul(pb[64:128, :], ones64[:], s1[:], start=True, stop=True)
                rc = wp.tile([P, 512], F32, name=f"rc_{j}", tag="rc", bufs=4)
                nc.vector.reciprocal_approx_fast(rc[:], pb[:])
                nc.vector.tensor_mul(ot[j][0:64, :], ppv0[0:64, :], rc[0:64, :])
                nc.vector.tensor_mul(ot[j][64:128, :], ppv1[0:64, :], rc[64:128, :])

            def outproj(j, t, on_scalar=False):
                # alternate the two HWDGE rings (SP / Activation) so the
                # output drain runs on two DMA queues in parallel
                ob = wp.tile([P, D], BF16, name=f"ob_{t}", tag="ob", bufs=8)
                for n in range(2):
                    po = pp.tile([P, 512], F32, name=f"po_{t}_{n}", tag="mx", bufs=2)
                    nc.tensor.matmul(
                        po[:], ot[j][:, ts(t - 4 * j, P)], wo[:, ts(n, 512)],
                        start=True, stop=True,
                    )
                    if on_scalar:
                        nc.scalar.copy(ob[:, ts(n, 512)], po[:])
                    else:
                        nc.vector.tensor_copy(ob[:, ts(n, 512)], po[:])
                nc.sync.dma_start(out_d[ts(t, P), :], ob[:])

            def final_slice(sl):
                """Normalize + out-project one 128-token slice of the last
                chunk, overlapped into its remaining diagonal i-iterations."""
                jf = NJ - 1
                fpv0, fpv1 = ppv[jf]
                c = ts(sl, P)
                s0 = wp.tile([1, P], BF16, name=f"s0f_{sl}", tag="s0f", bufs=4)
                s1 = wp.tile([1, P], BF16, name=f"s1f_{sl}", tag="s1f", bufs=4)
                nc.vector.tensor_copy(s0[:], fpv0[64:65, c])
                nc.vector.tensor_copy(s1[:], fpv1[64:65, c])
                pbf = pp.tile([P, P], F32, name=f"pbf_{sl}", tag="mx", bufs=2)
                nc.tensor.matmul(pbf[0:64, :], ones64[:], s0[:], start=True, stop=True)
                nc.tensor.matmul(pbf[64:128, :], ones64[:], s1[:], start=True, stop=True)
                rcf = wp.tile([P, P], F32, name=f"rcf_{sl}", tag="rcf", bufs=4)
                nc.vector.reciprocal_approx_fast(rcf[:], pbf[:])
                nc.vector.tensor_mul(ot[jf][0:64, c], fpv0[0:64, c], rcf[0:64, :])
                nc.vector.tensor_mul(ot[jf][64:128, c], fpv1[0:64, c], rcf[64:128, :])
                outproj(jf, 4 * jf + sl, on_scalar=True)

            # warm the PE p-state while the first DMAs land: ~10 dummy
            # matmuls on a memset tile keep the PE busy from t~0.3us so the
            # clock is ramped when real work arrives
            warm = cp.tile([P, 64], BF16, name="warm", tag="warm")
            nc.gpsimd.memset(warm[:], 0.0)
            for w in range(10):
                pw = pp.tile([P, 64], F32, name=f"pw{w}", tag="s", bufs=2)
                nc.tensor.matmul(pw[0:64, :], warm[0:64, :], warm[0:64, :],
                                 start=True, stop=True)

            # projections for chunk 0 up front
            proj_qk(0, "q")
            proj_qk(0, "k")
            vt0 = proj_vt(0)
            transp_v(0, vt0)

            for g in range(NJ):
                j = g
                nkj = 4 * (j + 1)
                ppv0 = pp.tile([65, 512], F32, name=f"ppv0_{j}", tag="ppv0", bufs=1)
                ppv1 = pp.tile([65, 512], F32, name=f"ppv1_{j}", tag="ppv1", bufs=1)
                ppv[j] = (ppv0, ppv1)

                # work units spread across this i-loop: projections for chunk
                # g+1, V tiles for chunk g+1, and the out-projection of the
                # already-normalized chunk g-2
                units = []
                if g + 1 < NJ:
                    units.append(lambda g=g: proj_qk(g + 1, "q"))
                    units.append(lambda g=g: proj_qk(g + 1, "k"))
                    vt_box = []
                    units.append(lambda g=g, b=vt_box: b.append(proj_vt(g + 1)))
                    units.append(lambda g=g, b=vt_box: transp_v(g + 1, b.pop()))
                if g >= 2:
                    for t in range(4 * (g - 2), 4 * (g - 2) + 4):
                        units.append(lambda t=t, g=g: outproj(g - 2, t))
                if g == NJ - 1:
                    for t in range(4 * (g - 1), 4 * (g - 1) + 4):
                        units.append(lambda t=t, g=g: outproj(g - 1, t))
                nu = len(units)
                slots = {}
                for u in range(nu):
                    slots.setdefault(min(nkj - 1, 1 + (u * nkj) // (nu + 1)), []).append(units[u])

                for i in range(nkj):
                    m = i - 4 * j
                    ps = pp.tile([P, 2, 512], F32, name=f"ps_{j}_{i}", tag="s", bufs=2)
                    c0 = 128 * m if m > 0 else 0
                    for h in range(2):
                        nc.tensor.matmul(
                            ps[:, h, c0:512],
                            kt[i // 4][ts(h, 64), ts(i % 4, P)],
                            qt[j][ts(h, 64), c0:512],
                            start=True, stop=True,
                        )
                    e = wp.tile([P, 2, 512], BF16, name=f"e_{j}_{i}", tag="e", bufs=8)
                    nc.scalar.activation(e[:, :, c0:512], ps[:, :, c0:512], EXP, scale=0.125)
                    if m >= 0:
                        for h in range(2):
                            nc.vector.tensor_mul(
                                e[:, h, c0:c0 + 128], e[:, h, c0:c0 + 128], maskt[:]
                            )
                    if i == 0 and j > 0:
                        normalize(j - 1)
                    # diagonal tiles only contribute to query columns >= c0;
                    # the rest of the accumulator is left untouched
                    nc.tensor.matmul(
                        ppv0[:, c0:512], v01[:, i, 0, :], e[:, 0, c0:512],
                        start=(i == 0), stop=(i == nkj - 1),
                        skip_group_check=True,
                    )
                    nc.tensor.matmul(
                        ppv1[:, c0:512], v01[:, i, 1, :], e[:, 1, c0:512],
                        start=(i == 0), stop=(i == nkj - 1),
                        skip_group_check=True,
                    )
                    # last chunk: each diagonal step completes one 128-token
                    # slice; normalize + out-project it while later diagonal
                    # steps still run
                    if j == NJ - 1 and m >= 1:
                        final_slice(m - 1)
                    for fn in slots.get(i, []):
                        fn()

            final_slice(3)
            ppv.pop(NJ - 1)

    nc.compile()
    return nc


def _pack_w(w_slice):
    """[HDC, D] weight slice -> transposed, chunked [P, KO, HDC] bf16."""
    wt = np.ascontiguousarray(w_slice.T)          # [D, HDC]
    return np.ascontiguousarray(
        wt.reshape(KO, P, HDC).transpose(1, 0, 2)
    ).astype(ml_dtypes.bfloat16)


def _make_in_maps(x, W_qkv, b_qkv, W_out, b_out):
    bf = ml_dtypes.bfloat16
    x = np.asarray(x, np.float32)
    W_qkv = np.asarray(W_qkv, np.float32)
    b_qkv = np.asarray(b_qkv, np.float32)
    W_out = np.asarray(W_out, np.float32)
    xt = np.ascontiguousarray(
        x.T.reshape(KO, P, NJ, 512).transpose(1, 2, 0, 3)
    ).astype(bf)
    in_maps = []
    for c in range(NCORES):
        r = slice(HDC * c, HDC * (c + 1))
        in_maps.append({
            "xt": xt,
            "wq": _pack_w(W_qkv[0 * D:1 * D][r]),
            "wk": _pack_w(W_qkv[1 * D:2 * D][r]),
            "wv": _pack_w(W_qkv[2 * D:3 * D][r]),
            "wo": np.ascontiguousarray(W_out[:, r].T).astype(bf),
            "bq": np.ascontiguousarray(b_qkv[0 * D:1 * D][r][:, None]).astype(np.float32),
            "bk": np.ascontiguousarray(b_qkv[1 * D:2 * D][r][:, None]).astype(np.float32),
            "bv": np.ascontiguousarray(b_qkv[2 * D:3 * D][r][:, None]).astype(np.float32),
        })
    return in_maps


_NC_CACHE = {}


def kernel(x, W_qkv, b_qkv, W_out, b_out):
    if "nc" not in _NC_CACHE:
        _NC_CACHE["nc"] = _build()
    nc = _NC_CACHE["nc"]
    in_maps = _make_in_maps(x, W_qkv, b_qkv, W_out, b_out)
    res = run_bass_kernel_spmd(nc, in_maps, core_ids=list(range(NCORES)))
    out = np.zeros((L, D), np.float32)
    for c in range(NCORES):
        out += res.results[c]["out"].astype(np.float32)
    out += np.asarray(b_out, np.float32)[None, :]
    return out


# revision 21
# speedup vs baseline: 1.0461x; 1.0298x over previous
"""Causal self-attention (L=4096, D=1024, 16 heads) on 8 TRN2 NeuronCores.

Sharding: tensor-parallel over heads — each core owns 2 heads (128 head-dims).
Per core:
  QT/KT = W @ x.T (+bias)          [128, L]   (head-dims on partitions)
  VT    = Wv @ x.T (+bias)         [128, L]   then PE-transposed to V tiles
  S.T   = K @ Q.T  (per head)      [k, q] blocks, causal-skipped
  E     = exp(S.T/8) * mask        (no max-subtraction: |logits| < ~3.1)
  O.T   = [V|1].T @ E              -> unnormalized head outputs + col-sums
  O.T  /= sums  (PE broadcast + DVE reciprocal)
  partial = O @ Wo_slice.T         [L, D]
Host: out = sum_c(partial_c) + b_out.

All matmuls bf16 with fp32 PSUM accumulation.  Diagonal k-tiles narrow the
score matmul, exp, and mask to the not-fully-masked columns; fully-masked
columns live in dedicated always-zero e tiles.  Bulk PSUM->SBUF copies run
on the Pool engine to keep the DVE queue short.  Emission interleaves
projection work for chunk g+1 into the attention i-loop of chunk g so the
PE always has fill work while ACT (the exp bottleneck) drains, and the
normalize/out-projection epilogue of chunk g-1 is deferred into chunk g's
loop head.
"""

import numpy as np
import ml_dtypes

import concourse.bass as bass
import concourse.mybir as mybir
import concourse.tile as tile
from concourse import bacc
from concourse.bass import ts
from concourse.bass_utils import run_bass_kernel_spmd

L, D = 4096, 1024
P = 128
NCORES = 8
HDC = 128          # head-dims per core (2 heads x 64)
KO = D // P        # 8 contraction chunks of the model dim
NJ = L // 512      # 8 q-chunks of 512
NK = L // P        # 32 k-chunks of 128
BF16 = mybir.dt.bfloat16
F32 = mybir.dt.float32
EXP = mybir.ActivationFunctionType.Exp


def _build():
    nc = bacc.Bacc("TRN2", target_bir_lowering=False)

    xt_d = nc.dram_tensor("xt", [P, NJ, KO, 512], BF16, kind="ExternalInput")
    wq_d = nc.dram_tensor("wq", [P, KO, HDC], BF16, kind="ExternalInput")
    wk_d = nc.dram_tensor("wk", [P, KO, HDC], BF16, kind="ExternalInput")
    wv_d = nc.dram_tensor("wv", [P, KO, HDC], BF16, kind="ExternalInput")
    wo_d = nc.dram_tensor("wo", [HDC, D], BF16, kind="ExternalInput")
    bq_d = nc.dram_tensor("bq", [HDC, 1], F32, kind="ExternalInput")
    bk_d = nc.dram_tensor("bk", [HDC, 1], F32, kind="ExternalInput")
    bv_d = nc.dram_tensor("bv", [HDC, 1], F32, kind="ExternalInput")
    out_d = nc.dram_tensor("out", [L, D], BF16, kind="ExternalOutput")

    # [128,128] causal triangle for the diagonal 128-col sub-block:
    # mask[p, c] = 1 if c >= p  (k-position p may attend-from query c)
    qi = np.arange(P)
    mask_np = (qi[None, :] >= qi[:, None]).astype(ml_dtypes.bfloat16)
    mask_d = nc.inline_tensor(np.ascontiguousarray(mask_np), name="maskc")
    ident_np = np.eye(P, dtype=ml_dtypes.bfloat16)
    ident_d = nc.inline_tensor(np.ascontiguousarray(ident_np), name="identc")
    ones64_d = nc.inline_tensor(np.ones((1, 64), ml_dtypes.bfloat16), name="ones64c")

    with tile.TileContext(nc) as tc:
        with (
            tc.tile_pool(name="const", bufs=1) as cp,
            tc.tile_pool(name="work", bufs=4) as wp,
            tc.tile_pool(name="psum", bufs=1, space="PSUM") as pp,
        ):
            # ---- weights first, k-sliced, so the very first matmul can
            # start after ~64KB instead of ~1.6MB ----
            wq = cp.tile([P, KO, HDC], BF16, name="wq_s", tag="wq_s")
            wk = cp.tile([P, KO, HDC], BF16, name="wk_s", tag="wk_s")
            wv = cp.tile([P, KO, HDC], BF16, name="wv_s", tag="wv_s")
            xta = cp.tile([P, NJ, KO, 512], BF16, name="xta", tag="xta")
            maskt = cp.tile([P, P], BF16, name="mask_s", tag="mask_s")
            ident = cp.tile([P, P], BF16, name="ident_s", tag="ident_s")
            wo = cp.tile([P, D], BF16, name="wo_s", tag="wo_s")
            bq = cp.tile([P, 1], F32, name="bq_s", tag="bq_s")
            bk = cp.tile([P, 1], F32, name="bk_s", tag="bk_s")
            bv = cp.tile([P, 1], F32, name="bv_s", tag="bv_s")
            ones64 = cp.tile([1, 64], BF16, name="ones64_s", tag="ones64_s")

            # interleave wq halves with the matching x halves of token
            # group 0: the first Q-projection matmuls gate on ~0.8MB.
            # Tiny constants load after the first half — each dma_start
            # costs ~0.6us of issue time on the sync queue.
            nc.sync.dma_start(wq[:, 0:4], wq_d[:, 0:4])
            nc.sync.dma_start(xta[:, 0, 0:4], xt_d[:, 0, 0:4])
            nc.sync.dma_start(wq[:, 4:8], wq_d[:, 4:8])
            nc.sync.dma_start(xta[:, 0, 4:8], xt_d[:, 0, 4:8])
            nc.sync.dma_start(bq[:], bq_d[:])
            nc.sync.dma_start(wk[:], wk_d[:])
            nc.sync.dma_start(bk[:], bk_d[:])
            nc.sync.dma_start(xta[:, 1], xt_d[:, 1])
            nc.sync.dma_start(wv[:], wv_d[:])
            nc.sync.dma_start(bv[:], bv_d[:])
            nc.sync.dma_start(ident[:], ident_d[:])
            nc.sync.dma_start(maskt[:], mask_d[:])
            nc.sync.dma_start(ones64[:], ones64_d[:])
            nc.sync.dma_start(xta[:, 2], xt_d[:, 2])
            nc.sync.dma_start(wo[:], wo_d[:])
            for jcol in range(3, NJ):
                nc.sync.dma_start(xta[:, jcol], xt_d[:, jcol])

            qt = [cp.tile([P, 512], BF16, name=f"qt{j}", tag=f"qt{j}") for j in range(NJ)]
            kt = [cp.tile([P, 512], BF16, name=f"kt{j}", tag=f"kt{j}") for j in range(NJ)]
            ot = [cp.tile([P, 512], BF16, name=f"ot{j}", tag=f"ot{j}") for j in range(NJ)]
            # all V tiles: [tok-tile, head, head-dim (64) + ones col]
            v01 = cp.tile([P, NK, 2, 65], BF16, name="v01", tag="v01")
            nc.gpsimd.memset(v01[:, :, :, 64:65], 1.0)

            ppv = {}  # j -> (ppv0, ppv1) accumulation psums kept until epilogue

            def proj_qk(g, which):
                """QT or KT projection for token chunk g."""
                w, b, dst, nm = (wq, bq, qt, "q") if which == "q" else (wk, bk, kt, "k")
                ps = pp.tile([P, 512], F32, name=f"ps{nm}{g}", tag="mx", bufs=2)
                for k in range(KO):
                    nc.tensor.matmul(
                        ps[:], w[:, k, :], xta[:, g, k, :],
                        start=(k == 0), stop=(k == KO - 1),
                    )
                nc.vector.tensor_scalar_add(dst[g][:], ps[:], b[:])

            def proj_vt(g):
                """VT projection for token chunk g -> vt SBUF tile [hd, 512]."""
                psv = pp.tile([P, 512], F32, name=f"psv{g}", tag="mx", bufs=2)
                for k in range(KO):
                    nc.tensor.matmul(
                        psv[:], wv[:, k, :], xta[:, g, k, :],
                        start=(k == 0), stop=(k == KO - 1),
                    )
                vt = wp.tile([P, 512], BF16, name=f"vt{g}", tag="vt", bufs=2)
                nc.vector.tensor_scalar_add(vt[:], psv[:], bv[:])
                return vt

            def transp_v(g, vt):
                """Transpose chunk g's VT into v01 token tiles (PE + Pool)."""
                tr = pp.tile([P, 4, P], BF16, name=f"tr{g}", tag="mx", bufs=2)
                for t in range(4):
                    nc.tensor.transpose(tr[:, t, :], vt[:, ts(t, P)], ident[:])
                for h in range(2):
                    nc.vector.tensor_copy(
                        v01[:, 4 * g:4 * g + 4, h, 0:64], tr[:, :, ts(h, 64)]
                    )

            def normalize(j):
                """Normalize chunk j's head outputs into ot[j]."""
                ppv0, ppv1 = ppv.pop(j)
                s0 = wp.tile([1, 512], BF16, name=f"s0_{j}", tag="s0", bufs=4)
                s1 = wp.tile([1, 512], BF16, name=f"s1_{j}", tag="s1", bufs=4)
                nc.vector.tensor_copy(s0[:], ppv0[64:65, :])
                nc.vector.tensor_copy(s1[:], ppv1[64:65, :])
                pb = pp.tile([P, 512], F32, name=f"pb_{j}", tag="mx", bufs=2)
                nc.tensor.matmul(pb[0:64, :], ones64[:], s0[:], start=True, stop=True)
                nc.tensor.matmul(pb[64:128, :], ones64[:], s1[:], start=True, stop=True)
                rc = wp.tile([P, 512], F32, name=f"rc_{j}", tag="rc", bufs=4)
                nc.vector.reciprocal_approx_fast(rc[:], pb[:])
                nc.vector.tensor_mul(ot[j][0:64, :], ppv0[0:64, :], rc[0:64, :])
                nc.vector.tensor_mul(ot[j][64:128, :], ppv1[0:64, :], rc[64:128, :])

            def outproj(j, t, on_scalar=False):
                # alternate the two HWDGE rings (SP / Activation) so the
                # output drain runs on two DMA queues in parallel
                ob = wp.tile([P, D], BF16, name=f"ob_{t}", tag="ob", bufs=8)
                for n in range(2):
                    po = pp.tile([P, 512], F32, name=f"po_{t}_{n}", tag="mx", bufs=2)
                    nc.tensor.matmul(
                        po[:], ot[j][:, ts(t - 4 * j, P)], wo[:, ts(n, 512)],
                        start=True, stop=True,
                    )
                    if on_scalar:
                        nc.scalar.copy(ob[:, ts(n, 512)], po[:])
                        # tail path: ship each half as soon as it is copied
                        nc.sync.dma_start(out_d[ts(t, P), ts(n, 512)], ob[:, ts(n, 512)])
                    else:
                        nc.vector.tensor_copy(ob[:, ts(n, 512)], po[:])
                if not on_scalar:
                    nc.sync.dma_start(out_d[ts(t, P), :], ob[:])

            def final_slice(sl):
                """Normalize + out-project one 128-token slice of the last
                chunk, overlapped into its remaining diagonal i-iterations."""
                jf = NJ - 1
                fpv0, fpv1 = ppv[jf]
                c = ts(sl, P)
                s0 = wp.tile([1, P], BF16, name=f"s0f_{sl}", tag="s0f", bufs=4)
                s1 = wp.tile([1, P], BF16, name=f"s1f_{sl}", tag="s1f", bufs=4)
                nc.vector.tensor_copy(s0[:], fpv0[64:65, c])
                nc.vector.tensor_copy(s1[:], fpv1[64:65, c])
                pbf = pp.tile([P, P], F32, name=f"pbf_{sl}", tag="mx", bufs=2)
                nc.tensor.matmul(pbf[0:64, :], ones64[:], s0[:], start=True, stop=True)
                nc.tensor.matmul(pbf[64:128, :], ones64[:], s1[:], start=True, stop=True)
                rcf = wp.tile([P, P], F32, name=f"rcf_{sl}", tag="rcf", bufs=4)
                nc.vector.reciprocal_approx_fast(rcf[:], pbf[:])
                nc.vector.tensor_mul(ot[jf][0:64, c], fpv0[0:64, c], rcf[0:64, :])
                nc.vector.tensor_mul(ot[jf][64:128, c], fpv1[0:64, c], rcf[64:128, :])
                outproj(jf, 4 * jf + sl, on_scalar=True)

            # warm the PE p-state while the first DMAs land: ~10 dummy
            # matmuls on a memset tile keep the PE busy from t~0.3us so the
            # clock is ramped when real work arrives
            warm = cp.tile([P, 64], BF16, name="warm", tag="warm")
            nc.gpsimd.memset(warm[:], 0.0)
            for w in range(10):
                pw = pp.tile([P, 64], F32, name=f"pw{w}", tag="s", bufs=2)
                nc.tensor.matmul(pw[0:64, :], warm[0:64, :], warm[0:64, :],
                                 start=True, stop=True)

            # projections for chunk 0 up front
            proj_qk(0, "q")
            proj_qk(0, "k")
            vt0 = proj_vt(0)
            transp_v(0, vt0)

            for g in range(NJ):
                j = g
                nkj = 4 * (j + 1)
                ppv0 = pp.tile([65, 512], F32, name=f"ppv0_{j}", tag="ppv0", bufs=1)
                ppv1 = pp.tile([65, 512], F32, name=f"ppv1_{j}", tag="ppv1", bufs=1)
                ppv[j] = (ppv0, ppv1)

                # work units spread across this i-loop: projections for chunk
                # g+1, V tiles for chunk g+1, and the out-projection of the
                # already-normalized chunk g-2
                units = []
                if g + 1 < NJ:
                    units.append(lambda g=g: proj_qk(g + 1, "q"))
                    units.append(lambda g=g: proj_qk(g + 1, "k"))
                    vt_box = []
                    units.append(lambda g=g, b=vt_box: b.append(proj_vt(g + 1)))
                    units.append(lambda g=g, b=vt_box: transp_v(g + 1, b.pop()))
                if g >= 2:
                    for t in range(4 * (g - 2), 4 * (g - 2) + 4):
                        units.append(lambda t=t, g=g: outproj(g - 2, t))
                if g == NJ - 1:
                    for t in range(4 * (g - 1), 4 * (g - 1) + 4):
                        units.append(lambda t=t, g=g: outproj(g - 1, t))
                nu = len(units)
                slots = {}
                for u in range(nu):
                    slots.setdefault(min(nkj - 1, 1 + (u * nkj) // (nu + 1)), []).append(units[u])

                es = {}

                def emit_se(i):
                    """Score matmuls + exp + mask for k-tile i of chunk j."""
                    m = i - 4 * j
                    c0 = 128 * m if m > 0 else 0
                    ps = pp.tile([P, 2, 512], F32, name=f"ps_{j}_{i}", tag="s", bufs=2)
                    for h in range(2):
                        nc.tensor.matmul(
                            ps[:, h, c0:512],
                            kt[i // 4][ts(h, 64), ts(i % 4, P)],
                            qt[j][ts(h, 64), c0:512],
                            start=True, stop=True,
                        )
                    e = wp.tile([P, 2, 512], BF16, name=f"e_{j}_{i}", tag="e", bufs=8)
                    nc.scalar.activation(e[:, :, c0:512], ps[:, :, c0:512], EXP, scale=0.125)
                    if m >= 0:
                        for h in range(2):
                            nc.vector.tensor_mul(
                                e[:, h, c0:c0 + 128], e[:, h, c0:c0 + 128], maskt[:]
                            )
                    es[i] = e

                # scores/exp run one k-tile ahead of the AV consumption so
                # the PE bypass window always holds ready score work while
                # an AV waits on its exp (or on the ppv WAR at chunk start)
                emit_se(0)
                for i in range(nkj):
                    m = i - 4 * j
                    c0 = 128 * m if m > 0 else 0
                    if i + 1 < nkj:
                        emit_se(i + 1)
                    e = es.pop(i)
                    if i == 0 and j > 0:
                        normalize(j - 1)
                    # diagonal tiles only contribute to query columns >= c0;
                    # the rest of the accumulator is left untouched
                    nc.tensor.matmul(
                        ppv0[:, c0:512], v01[:, i, 0, :], e[:, 0, c0:512],
                        start=(i == 0), stop=(i == nkj - 1),
                        skip_group_check=True,
                    )
                    nc.tensor.matmul(
                        ppv1[:, c0:512], v01[:, i, 1, :], e[:, 1, c0:512],
                        start=(i == 0), stop=(i == nkj - 1),
                        skip_group_check=True,
                    )
                    # last chunk: each diagonal step completes one 128-token
                    # slice; normalize + out-project it while later diagonal
                    # steps still run
                    if j == NJ - 1 and m >= 1:
                        final_slice(m - 1)
                    for fn in slots.get(i, []):
                        fn()

            final_slice(3)
            ppv.pop(NJ - 1)

    nc.compile()
    return nc


def _pack_w(w_slice):
    """[HDC, D] weight slice -> transposed, chunked [P, KO, HDC] bf16."""
    wt = np.ascontiguousarray(w_slice.T)          # [D, HDC]
    return np.ascontiguousarray(
        wt.reshape(KO, P, HDC).transpose(1, 0, 2)
    ).astype(ml_dtypes.bfloat16)


def _make_in_maps(x, W_qkv, b_qkv, W_out, b_out):
    bf = ml_dtypes.bfloat16
    x = np.asarray(x, np.float32)
    W_qkv = np.asarray(W_qkv, np.float32)
    b_qkv = np.asarray(b_qkv, np.float32)
    W_out = np.asarray(W_out, np.float32)
    xt = np.ascontiguousarray(
        x.T.reshape(KO, P, NJ, 512).transpose(1, 2, 0, 3)
    ).astype(bf)
    in_maps = []
    for c in range(NCORES):
        r = slice(HDC * c, HDC * (c + 1))
        in_maps.append({
            "xt": xt,
            "wq": _pack_w(W_qkv[0 * D:1 * D][r]),
            "wk": _pack_w(W_qkv[1 * D:2 * D][r]),
            "wv": _pack_w(W_qkv[2 * D:3 * D][r]),
            "wo": np.ascontiguousarray(W_out[:, r].T).astype(bf),
            "bq": np.ascontiguousarray(b_qkv[0 * D:1 * D][r][:, None]).astype(np.float32),
            "bk": np.ascontiguousarray(b_qkv[1 * D:2 * D][r][:, None]).astype(np.float32),
            "bv": np.ascontiguousarray(b_qkv[2 * D:3 * D][r][:, None]).astype(np.float32),
        })
    return in_maps


_NC_CACHE = {}


def kernel(x, W_qkv, b_qkv, W_out, b_out):
    if "nc" not in _NC_CACHE:
        _NC_CACHE["nc"] = _build()
    nc = _NC_CACHE["nc"]
    in_maps = _make_in_maps(x, W_qkv, b_qkv, W_out, b_out)
    res = run_bass_kernel_spmd(nc, in_maps, core_ids=list(range(NCORES)))
    out = np.zeros((L, D), np.float32)
    for c in range(NCORES):
        out += res.results[c]["out"].astype(np.float32)
    out += np.asarray(b_out, np.float32)[None, :]
    return out
